# revision 1
# baseline (speedup 1.0000x reference)
"""Trainium2 Bass kernel for block-local causal multi-head attention.

Problem (hardcoded): x [4, 4096, 1024] f32, 4x [1024,1024] projection
weights + biases. Sequence is split into independent causal blocks of 256.
B*nb = 64 blocks -> 8 blocks per core across 8 NeuronCores (data parallel,
weights replicated, no collectives).

Dataflow (per core, feature-major / "transposed" so no input transposes):
  - host ships xT = x_shard.T [1024, 2048] bf16
  - Q^T, K^T = W.T @ xT  [1024, 2048] (feature-major)
  - V natural = xT.T @ Wv [2048, 1024], stored 3D [128, 16 heads, 65] with a
    ones column appended per head (col 64)
  - scores TRANSPOSED: S^T[k, q] = (K^T slice).T @ (Q^T slice) per
    (block, head) -- [sk, sq] layout, no P transposes needed
  - mask + exp(S^T/8) -> E^T bf16 (single ACT op per head, no accum)
  - PV: O_un[d|ones, q] = [V|1].T @ E^T -- row 64 = softmax denominator
  - normalization: rr = 1/dsum (DVE recip of PSUM row), rank-1 broadcast
    R = ones[1,64].T @ rr on PE, ats = O_un * R on DVE
  - y^T = Wo.T @ attn^T -> [1024, 2048] f32; host transposes back.

Scheduling: dense GEMM work (QKV projections of the NEXT pair, output
projections of PREVIOUS blocks) is kept in a FIFO of "thunks" and emitted
interleaved into the per-head attention loop, so the tensor engine always
has streaming work while softmax round-trips (DVE/ACT) are in flight.

Biases: bq/bk applied as fused per-partition ACT bias on PSUM->SBUF
evacuation; bv is folded into bo on host (softmax rows sum to 1), bo applied
at the output-projection evacuation.
"""

import sys

if "/opt/trn_rl_repo" not in sys.path:
    sys.path.insert(0, "/opt/trn_rl_repo")

import ml_dtypes
import numpy as np

import concourse.bass as bass
import concourse.mybir as mybir
import concourse.tile as tile

N_CORES = 8
D = 1024
BLK = 256
NH = 16
DH = 64
B, S = 4, 4096
N_BLOCKS = B * (S // BLK)  # 64
BLOCKS_PER_CORE = N_BLOCKS // N_CORES  # 8
SEQ = BLOCKS_PER_CORE * BLK  # 2048 seq positions per core
N_PAIRS = BLOCKS_PER_CORE // 2  # 4 pairs of blocks (512 seq cols each)
MASK_NEG = -1.0e4  # pre-scale additive mask; exp(0.125 * -1e4) == 0.0

BF16 = ml_dtypes.bfloat16
AF = mybir.ActivationFunctionType
dt = mybir.dt

_cache = {}


def _psum_pools(tc):
    import contextlib

    @contextlib.contextmanager
    def mgr():
        with (
            tc.tile_pool(name="pdense", bufs=2, space="PSUM") as pdense,
            tc.tile_pool(name="ps_s", bufs=3, space="PSUM") as ps_s,
            tc.tile_pool(name="ps_o", bufs=2, space="PSUM") as ps_o,
            tc.tile_pool(name="ps_r", bufs=1, space="PSUM") as ps_r,
        ):
            yield pdense, ps_s, ps_o, ps_r

    return mgr()


def _legalize_waits(nc, max_waits=1):
    """This environment's walrus build rejects instructions with more than
    one sync-wait command ("Too many sync wait commands"). Split extra waits
    onto same-engine NoOps inserted immediately before the instruction —
    semantically identical (engine streams are in-order)."""
    fn = nc.m.functions[0]
    k = 0
    for blk in fn.blocks:
        insts = blk.instructions
        if not any(
            i.sync_info is not None and len(i.sync_info.on_wait) > max_waits
            for i in insts
        ):
            continue
        new = []
        for inst in insts:
            si = inst.sync_info
            if si is not None and len(si.on_wait) > max_waits:
                waits = list(si.on_wait)
                for w in waits[:-max_waits]:
                    k += 1
                    new.append(
                        mybir.InstNoOp(
                            name=f"I-wsplit-{k}",
                            engine=inst.engine,
                            sync_info=mybir.SyncInfo(on_wait=[w], on_update=[]),
                        )
                    )
                inst.sync_info = mybir.SyncInfo(
                    on_wait=waits[-max_waits:], on_update=list(si.on_update)
                )
            new.append(inst)
        blk.instructions = new


def _build_nc(repeat=1, legalize=True):
    nc = bass.Bass(
        "TRN2", target_bir_lowering=True, debug=False, enable_asserts=False
    )

    xT = nc.dram_tensor("xT", [D, SEQ], dt.bfloat16, kind="ExternalInput").ap()
    wq = nc.dram_tensor("wq", [D, D], dt.bfloat16, kind="ExternalInput").ap()
    wk = nc.dram_tensor("wk", [D, D], dt.bfloat16, kind="ExternalInput").ap()
    wv = nc.dram_tensor("wv", [D, D], dt.bfloat16, kind="ExternalInput").ap()
    wo = nc.dram_tensor("wo", [D, D], dt.bfloat16, kind="ExternalInput").ap()
    bqt = nc.dram_tensor("bqt", [128, 8], dt.float32, kind="ExternalInput").ap()
    bkt = nc.dram_tensor("bkt", [128, 8], dt.float32, kind="ExternalInput").ap()
    bot = nc.dram_tensor("bot", [128, 8], dt.float32, kind="ExternalInput").ap()
    msk = nc.dram_tensor("msk", [128, 384], dt.bfloat16, kind="ExternalInput").ap()
    yT = nc.dram_tensor("yT", [D, SEQ], dt.float32, kind="ExternalOutput").ap()

    with tile.TileContext(nc) as tc:
        with (
            tc.tile_pool(name="const", bufs=1) as constp,
            tc.tile_pool(name="xw", bufs=1) as xwp,
            tc.tile_pool(name="qkv", bufs=2) as qkvp,
            tc.tile_pool(name="attn", bufs=2) as attnp,
            tc.tile_pool(name="atp", bufs=4) as atp,
            tc.tile_pool(name="yp", bufs=4) as yp,
        ):
            mask_sb = constp.tile([128, 384], dt.bfloat16, name="mask_sb")
            nc.sync.dma_start(out=mask_sb[:], in_=msk)
            bq_sb = constp.tile([128, 8], dt.float32, name="bq_sb")
            nc.sync.dma_start(out=bq_sb[:], in_=bqt)
            bk_sb = constp.tile([128, 8], dt.float32, name="bk_sb")
            nc.sync.dma_start(out=bk_sb[:], in_=bkt)
            bo_sb = constp.tile([128, 8], dt.float32, name="bo_sb")
            nc.sync.dma_start(out=bo_sb[:], in_=bot)
            ones1 = constp.tile([1, 64], dt.bfloat16, name="ones1")
            nc.vector.memset(ones1[:], 1.0)

            for _rep in range(repeat):
                import contextlib

                ctx_stack = contextlib.ExitStack()
                # ---- input DMAs, in phase-0 consumption order
                xts, wqs, wks, wvs, wos = [], [], [], [], []
                for k in range(8):
                    w = xwp.tile([128, D], dt.bfloat16, name=f"wq{k}", tag=f"wq{k}")
                    nc.sync.dma_start(out=w[:], in_=wq[k * 128 : (k + 1) * 128, :])
                    wqs.append(w)
                    t = xwp.tile([128, SEQ], dt.bfloat16, name=f"xt{k}", tag=f"xt{k}")
                    nc.sync.dma_start(out=t[:], in_=xT[k * 128 : (k + 1) * 128, :])
                    xts.append(t)
                for nm, wap, lst in (("wk", wk, wks), ("wv", wv, wvs), ("wo", wo, wos)):
                    for k in range(8):
                        w = xwp.tile(
                            [128, D], dt.bfloat16, name=f"{nm}{k}", tag=f"{nm}{k}"
                        )
                        nc.sync.dma_start(out=w[:], in_=wap[k * 128 : (k + 1) * 128, :])
                        lst.append(w)

                # per-pair SBUF tile allocators -------------------------------
                def alloc_qk(p):
                    qts = [
                        qkvp.tile([128, 512], dt.bfloat16, name=f"q{m}", tag=f"q{m}")
                        for m in range(8)
                    ]
                    kts = [
                        qkvp.tile([128, 512], dt.bfloat16, name=f"k{m}", tag=f"k{m}")
                        for m in range(8)
                    ]
                    return qts, kts

                def alloc_v(p):
                    vts = []
                    for st in range(4):
                        vt = qkvp.tile(
                            [128, NH, 65], dt.bfloat16, name=f"v{st}", tag=f"v{st}"
                        )
                        nc.vector.memset(vt[:, :, 64:65], 1.0)
                        vts.append(vt)
                    return vts

                def alloc_ats(b):
                    return [
                        atp.tile([128, 256], dt.bfloat16, name=f"at{k}", tag=f"at{k}")
                        for k in range(8)
                    ]

                # dense-work FIFOs: hard = QKV (schedule-critical),
                # soft = output projections (deferrable fillers)
                fifo_hard = []
                fifo_soft = []

                psum_pools = ctx_stack.enter_context(
                    _psum_pools(tc)
                )
                pdense, ps_s, ps_o, ps_r = psum_pools

                def push_qkv(p, qts, kts, vts):
                    pc0 = p * 512

                    def qk_thunk(wlist, b_sb, dst, m):
                        def run():
                            ps = pdense.tile(
                                [128, 512], dt.float32, name=f"pd_{m}", tag="dense"
                            )
                            for k in range(8):
                                nc.tensor.matmul(
                                    ps[:],
                                    wlist[k][:, m * 128 : (m + 1) * 128],
                                    xts[k][:, pc0 : pc0 + 512],
                                    start=(k == 0),
                                    stop=(k == 7),
                                )
                            nc.scalar.activation(
                                dst[m][:], ps[:], AF.Identity, bias=b_sb[:, m : m + 1]
                            )

                        return run

                    def v_thunk(st, ch):
                        def run():
                            ps = pdense.tile(
                                [128, 512], dt.float32, name=f"pv_{st}{ch}", tag="dense"
                            )
                            for k in range(8):
                                nc.tensor.matmul(
                                    ps[:],
                                    xts[k][:, pc0 + st * 128 : pc0 + (st + 1) * 128],
                                    wvs[k][:, ch * 512 : (ch + 1) * 512],
                                    start=(k == 0),
                                    stop=(k == 7),
                                )
                            nc.vector.tensor_copy(
                                vts[st][:, ch * 8 : (ch + 1) * 8, 0:64],
                                ps[:].rearrange("p (h c) -> p h c", h=8),
                            )

                        return run

                    for m in range(8):
                        fifo_hard.append(qk_thunk(wqs, bq_sb, qts, m))
                    for m in range(8):
                        fifo_hard.append(qk_thunk(wks, bk_sb, kts, m))
                    for st in range(4):
                        for ch in range(2):
                            fifo_hard.append(v_thunk(st, ch))

                def emit_op(b, ats, m):
                    bc0 = b * 256
                    ps = pdense.tile(
                        [128, 256], dt.float32, name=f"po_{m}", tag="dense"
                    )
                    for k in range(8):
                        nc.tensor.matmul(
                            ps[:],
                            wos[k][:, m * 128 : (m + 1) * 128],
                            ats[k][:],
                            start=(k == 0),
                            stop=(k == 7),
                        )
                    yt = yp.tile([128, 256], dt.float32, name=f"yt{m}", tag="yt")
                    nc.vector.tensor_scalar_add(yt[:], ps[:], bo_sb[:, m : m + 1])
                    nc.sync.dma_start(
                        out=yT[m * 128 : (m + 1) * 128, bc0 : bc0 + 256],
                        in_=yt[:],
                    )

                def push_op(b, ats):
                    def op_thunk(m):
                        return lambda: emit_op(b, ats, m)

                    for m in range(8):
                        fifo_soft.append(op_thunk(m))

                def pop(reserve=0):
                    if fifo_hard:
                        fifo_hard.pop(0)()
                    elif len(fifo_soft) > reserve:
                        fifo_soft.pop(0)()

                # ---- phase 0: k-outer QKV for pair 0 (fills the DMA ramp).
                # PSUM slots are bank-granular, so phase-0 borrows one
                # generation of every main-pool tag (2+3+2+1 = 8 banks).
                qk0 = alloc_qk(0)
                vts0 = alloc_v(0)

                def alloc8(nm):
                    return [
                        pdense.tile([128, 512], dt.float32, name=f"{nm}d0", tag="dense"),
                        pdense.tile([128, 512], dt.float32, name=f"{nm}d1", tag="dense"),
                        ps_s.tile([128, 512], dt.float32, name=f"{nm}s0", tag="s"),
                        ps_s.tile([128, 512], dt.float32, name=f"{nm}s1", tag="s"),
                        ps_s.tile([128, 512], dt.float32, name=f"{nm}s2", tag="s"),
                        ps_o.tile([128, 512], dt.float32, name=f"{nm}o0", tag="o"),
                        ps_o.tile([128, 512], dt.float32, name=f"{nm}o1", tag="o"),
                        ps_r.tile([128, 512], dt.float32, name=f"{nm}r0", tag="r"),
                    ]

                if True:
                    for wlist, b_sb, dst in ((wqs, bq_sb, qk0[0]), (wks, bk_sb, qk0[1])):
                        pss = alloc8("q" if wlist is wqs else "k")
                        for k in range(8):
                            for m in range(8):
                                nc.tensor.matmul(
                                    pss[m][:],
                                    wlist[k][:, m * 128 : (m + 1) * 128],
                                    xts[k][:, 0:512],
                                    start=(k == 0),
                                    stop=(k == 7),
                                )
                        for m in range(8):
                            nc.scalar.activation(
                                dst[m][:],
                                pss[m][:],
                                AF.Identity,
                                bias=b_sb[:, m : m + 1],
                            )
                    pss = alloc8("v")
                    for k in range(8):
                        for st in range(4):
                            for ch in range(2):
                                nc.tensor.matmul(
                                    pss[st * 2 + ch][:],
                                    xts[k][:, st * 128 : (st + 1) * 128],
                                    wvs[k][:, ch * 512 : (ch + 1) * 512],
                                    start=(k == 0),
                                    stop=(k == 7),
                                )
                    for st in range(4):
                        for ch in range(2):
                            nc.vector.tensor_copy(
                                vts0[st][:, ch * 8 : (ch + 1) * 8, 0:64],
                                pss[st * 2 + ch][:].rearrange("p (h c) -> p h c", h=8),
                            )

                # ---- main loop
                if True:
                    cur_qk, cur_v = qk0, vts0
                    for p in range(N_PAIRS):
                        if p < N_PAIRS - 1:
                            nqk = alloc_qk(p + 1)
                            nv = alloc_v(p + 1)
                            push_qkv(p + 1, nqk[0], nqk[1], nv)
                        qts, kts = cur_qk
                        vts = cur_v

                        for beta in range(2):
                            b = 2 * p + beta
                            bc0 = beta * 256
                            ats = alloc_ats(b)
                            v0 = vts[2 * beta]
                            v1 = vts[2 * beta + 1]

                            # software-pipelined heads: S(t) ... PV(t-1)
                            es = {}
                            ouns = {}

                            def s_phase(h):
                                ht, hp = h // 2, (h % 2) * 64
                                s_ = ps_s.tile(
                                    [128, 384], dt.float32, name=f"s{h}", tag="s"
                                )
                                nc.tensor.matmul(
                                    s_[:, 0:256],
                                    kts[ht][hp : hp + 64, bc0 : bc0 + 128],
                                    qts[ht][hp : hp + 64, bc0 : bc0 + 256],
                                    start=True,
                                    stop=True,
                                )
                                nc.tensor.matmul(
                                    s_[:, 256:384],
                                    kts[ht][hp : hp + 64, bc0 + 128 : bc0 + 256],
                                    qts[ht][hp : hp + 64, bc0 + 128 : bc0 + 256],
                                    start=True,
                                    stop=True,
                                )
                                e_ = attnp.tile(
                                    [128, 384], dt.bfloat16, name=f"e{h}", tag="e",
                                    bufs=4,
                                )
                                nc.scalar.activation(
                                    e_[:], s_[:], AF.Exp, scale=0.125
                                )
                                # zero the causally-invalid region (bf16 2x DVE)
                                nc.vector.tensor_tensor(
                                    e_[:], e_[:], mask_sb[:], mybir.AluOpType.mult
                                )
                                es[h] = e_

                            def pv_phase(t):
                                rrs = []
                                for j in range(2):
                                    h = 2 * t + j
                                    e_ = es.pop(h)
                                    o_ = ps_o.tile(
                                        [65, 256], dt.float32, name=f"o{h}", tag="o"
                                    )
                                    nc.tensor.matmul(
                                        o_[:, 0:128],
                                        v0[:, h, :],
                                        e_[:, 0:128],
                                        start=True,
                                        stop=True,
                                    )
                                    nc.tensor.matmul(
                                        o_[:, 128:256],
                                        v0[:, h, :],
                                        e_[:, 128:256],
                                        start=True,
                                        stop=False,
                                    )
                                    nc.tensor.matmul(
                                        o_[:, 128:256],
                                        v1[:, h, :],
                                        e_[:, 256:384],
                                        start=False,
                                        stop=True,
                                    )
                                    # rr = 1/dsum as exp(-ln(dsum)) on ACT
                                    # (DVE reciprocal is ~6.7ns/elem microcode)
                                    lnv = attnp.tile(
                                        [1, 256], dt.float32, name=f"ln{j}",
                                        tag=f"ln{j}",
                                    )
                                    nc.scalar.activation(
                                        lnv[:], o_[64:65, :], AF.Ln
                                    )
                                    rr = attnp.tile(
                                        [1, 256], dt.bfloat16, name=f"rr{j}",
                                        tag=f"rr{j}",
                                    )
                                    nc.scalar.activation(
                                        rr[:], lnv[:], AF.Exp, scale=-1.0
                                    )
                                    rrs.append(rr)
                                    ouns[h] = o_
                                r2 = ps_r.tile(
                                    [128, 256], dt.float32, name=f"r2_{t}", tag="r"
                                )
                                nc.tensor.matmul(
                                    r2[0:64, :], ones1[:], rrs[0][:],
                                    start=True, stop=True,
                                )
                                nc.tensor.matmul(
                                    r2[64:128, :], ones1[:], rrs[1][:],
                                    start=True, stop=True,
                                )
                                r2sb = attnp.tile(
                                    [128, 256], dt.float32, name=f"r2sb{t}", tag="r2sb"
                                )
                                nc.vector.tensor_copy(r2sb[:], r2[:])
                                for j in range(2):
                                    h = 2 * t + j
                                    o_ = ouns.pop(h)
                                    nc.vector.tensor_tensor(
                                        ats[t][j * 64 : (j + 1) * 64, :],
                                        o_[0:64, :],
                                        r2sb[j * 64 : (j + 1) * 64, :],
                                        mybir.AluOpType.mult,
                                    )

                            rsv = 8 if b < 2 * N_PAIRS - 2 else 0
                            for t in range(9):
                                if t < 8:
                                    s_phase(2 * t)
                                    s_phase(2 * t + 1)
                                pop(rsv)
                                if t >= 1:
                                    pv_phase(t - 1)
                                pop(rsv)

                            push_op(b, ats)
                        cur_qk, cur_v = (nqk, nv) if p < N_PAIRS - 1 else (None, None)

                    while fifo_hard or fifo_soft:
                        pop()
                ctx_stack.close()
    if legalize:
        _legalize_waits(nc)
    return nc


def get_nc():
    if "nc" not in _cache:
        _cache["nc"] = _build_nc()
    return _cache["nc"]


def make_in_maps(x, Wq, bq, Wk, bk, Wv, bv, Wo, bo):
    """Host-side sharding/packing. Returns list of 8 per-core input dicts."""
    x = np.asarray(x, np.float32)
    Wq, Wk, Wv, Wo = (np.asarray(w, np.float32) for w in (Wq, Wk, Wv, Wo))
    bq, bk, bv, bo = (np.asarray(b, np.float32) for b in (bq, bk, bv, bo))

    # softmax rows sum to 1 -> attn @ (V + bv) = attn @ V + bv; fold into bo
    bo2 = bo + bv @ Wo

    wq_b, wk_b, wv_b, wo_b = (w.astype(BF16) for w in (Wq, Wk, Wv, Wo))
    bqt = np.ascontiguousarray(bq.reshape(8, 128).T)
    bkt = np.ascontiguousarray(bk.reshape(8, 128).T)
    bot = np.ascontiguousarray(bo2.reshape(8, 128).T)
    # S^T layout: rows = k, cols = q; invalid where k > q (strictly below
    # diag). 0/1 multiplicative mask applied to E after exp, bf16.
    tri01 = np.triu(np.ones((128, 128), np.float32))
    msk = np.concatenate(
        [tri01, np.ones((128, 128), np.float32), tri01], axis=1
    ).astype(BF16)

    xb = x.reshape(N_BLOCKS, BLK, D)
    in_maps = []
    for c in range(N_CORES):
        xc = xb[c * BLOCKS_PER_CORE : (c + 1) * BLOCKS_PER_CORE].reshape(SEQ, D)
        xTc = np.ascontiguousarray(xc.T.astype(BF16))
        in_maps.append(
            {
                "xT": xTc,
                "wq": wq_b,
                "wk": wk_b,
                "wv": wv_b,
                "wo": wo_b,
                "bqt": bqt,
                "bkt": bkt,
                "bot": bot,
                "msk": msk,
            }
        )
    return in_maps


def assemble_output(yT_list):
    """yT_list: per-core [1024, 2048] f32 -> full [4, 4096, 1024] f32."""
    y = np.empty((N_BLOCKS, BLK, D), np.float32)
    for c in range(N_CORES):
        y[c * BLOCKS_PER_CORE : (c + 1) * BLOCKS_PER_CORE] = (
            yT_list[c].T.reshape(BLOCKS_PER_CORE, BLK, D)
        )
    return np.ascontiguousarray(y.reshape(B, S, D))


def kernel(x, Wq, bq, Wk, bk, Wv, bv, Wo, bo):
    from concourse.bass_utils import run_bass_kernel_spmd

    in_maps = make_in_maps(x, Wq, bq, Wk, bk, Wv, bv, Wo, bo)
    nc = get_nc()
    res = run_bass_kernel_spmd(nc, in_maps, list(range(N_CORES)))
    return assemble_output([res.results[c]["yT"] for c in range(N_CORES)])



# revision 3
# speedup vs baseline: 1.0258x; 1.0258x over previous
"""Trainium2 Bass kernel for block-local causal multi-head attention.

Problem (hardcoded): x [4, 4096, 1024] f32, 4x [1024,1024] projection
weights + biases. Sequence is split into independent causal blocks of 256.
B*nb = 64 blocks -> 8 blocks per core across 8 NeuronCores (data parallel,
weights replicated, no collectives).

Dataflow (per core, feature-major / "transposed" so no input transposes):
  - host ships xT = x_shard.T [1024, 2048] bf16
  - Q^T, K^T = W.T @ xT  [1024, 2048] (feature-major)
  - V natural = xT.T @ Wv [2048, 1024], stored 3D [128, 16 heads, 65] with a
    ones column appended per head (col 64)
  - scores TRANSPOSED: S^T[k, q] = (K^T slice).T @ (Q^T slice) per
    (block, head) -- [sk, sq] layout, no P transposes needed
  - mask + exp(S^T/8) -> E^T bf16 (single ACT op per head, no accum)
  - PV: both heads of a pair into ONE [65, 512] PSUM bank:
    O_un[d|ones, q] = [V|1].T @ E^T -- row 64 = softmax denominators for
    both heads, so rr = exp(-ln(d)) needs one [1,512] Ln + Exp pair per
    head-pair (ACT cost is free-dim-proportional, so batching the two
    [1,256] rows into [1,512] halves the per-op startup overhead)
  - normalization: rank-1 broadcast R = ones[1,64].T @ rr on PE, ats =
    O_un * R on DVE
  - y^T = Wo.T @ attn^T per 2-block pair (N=512) -> [1024, 2048] bf16;
    host upcasts and transposes back.

Scheduling: dense GEMM work (QKV projections of the NEXT pair, output
projections of PREVIOUS pairs) is kept in a FIFO of "thunks" and emitted
interleaved into the per-head attention loop, so the tensor engine always
has streaming work while softmax round-trips (DVE/ACT) are in flight.

Biases: bq/bk applied as fused per-partition ACT bias on PSUM->SBUF
evacuation; bv is folded into bo on host (softmax rows sum to 1), bo applied
at the output-projection evacuation.
"""

import sys

if "/opt/trn_rl_repo" not in sys.path:
    sys.path.insert(0, "/opt/trn_rl_repo")

import ml_dtypes
import numpy as np

import concourse.bass as bass
import concourse.mybir as mybir
import concourse.tile as tile

N_CORES = 8
D = 1024
BLK = 256
NH = 16
DH = 64
B, S = 4, 4096
N_BLOCKS = B * (S // BLK)  # 64
BLOCKS_PER_CORE = N_BLOCKS // N_CORES  # 8
SEQ = BLOCKS_PER_CORE * BLK  # 2048 seq positions per core
N_PAIRS = BLOCKS_PER_CORE // 2  # 4 pairs of blocks (512 seq cols each)

BF16 = ml_dtypes.bfloat16
AF = mybir.ActivationFunctionType
dt = mybir.dt

_cache = {}


def _psum_pools(tc):
    import contextlib

    @contextlib.contextmanager
    def mgr():
        with (
            tc.tile_pool(name="pdense", bufs=2, space="PSUM") as pdense,
            tc.tile_pool(name="ps_s", bufs=3, space="PSUM") as ps_s,
            tc.tile_pool(name="ps_o", bufs=2, space="PSUM") as ps_o,
            tc.tile_pool(name="ps_r", bufs=1, space="PSUM") as ps_r,
        ):
            yield pdense, ps_s, ps_o, ps_r

    return mgr()


def _legalize_waits(nc, max_waits=1):
    """This environment's walrus build rejects instructions with more than
    one sync-wait command ("Too many sync wait commands"). Split extra waits
    onto same-engine NoOps inserted immediately before the instruction —
    semantically identical (engine streams are in-order)."""
    fn = nc.m.functions[0]
    k = 0
    for blk in fn.blocks:
        insts = blk.instructions
        if not any(
            i.sync_info is not None and len(i.sync_info.on_wait) > max_waits
            for i in insts
        ):
            continue
        new = []
        for inst in insts:
            si = inst.sync_info
            if si is not None and len(si.on_wait) > max_waits:
                waits = list(si.on_wait)
                for w in waits[:-max_waits]:
                    k += 1
                    new.append(
                        mybir.InstNoOp(
                            name=f"I-wsplit-{k}",
                            engine=inst.engine,
                            sync_info=mybir.SyncInfo(on_wait=[w], on_update=[]),
                        )
                    )
                inst.sync_info = mybir.SyncInfo(
                    on_wait=waits[-max_waits:], on_update=list(si.on_update)
                )
            new.append(inst)
        blk.instructions = new


def _build_nc(repeat=1, legalize=True):
    nc = bass.Bass(
        "TRN2", target_bir_lowering=True, debug=False, enable_asserts=False
    )

    xT = nc.dram_tensor("xT", [D, SEQ], dt.bfloat16, kind="ExternalInput").ap()
    wq = nc.dram_tensor("wq", [D, D], dt.bfloat16, kind="ExternalInput").ap()
    wk = nc.dram_tensor("wk", [D, D], dt.bfloat16, kind="ExternalInput").ap()
    wv = nc.dram_tensor("wv", [D, D], dt.bfloat16, kind="ExternalInput").ap()
    wo = nc.dram_tensor("wo", [D, D], dt.bfloat16, kind="ExternalInput").ap()
    bqt = nc.dram_tensor("bqt", [128, 8], dt.float32, kind="ExternalInput").ap()
    bkt = nc.dram_tensor("bkt", [128, 8], dt.float32, kind="ExternalInput").ap()
    bot = nc.dram_tensor("bot", [128, 8], dt.float32, kind="ExternalInput").ap()
    msk = nc.dram_tensor("msk", [128, 384], dt.bfloat16, kind="ExternalInput").ap()
    yT = nc.dram_tensor("yT", [D, SEQ], dt.bfloat16, kind="ExternalOutput").ap()

    with tile.TileContext(nc) as tc:
        with (
            tc.tile_pool(name="const", bufs=1) as constp,
            tc.tile_pool(name="xw", bufs=1) as xwp,
            tc.tile_pool(name="qkv", bufs=2) as qkvp,
            tc.tile_pool(name="attn", bufs=2) as attnp,
            tc.tile_pool(name="atp", bufs=2) as atp,
            tc.tile_pool(name="yp", bufs=4) as yp,
        ):
            mask_sb = constp.tile([128, 384], dt.bfloat16, name="mask_sb")
            nc.sync.dma_start(out=mask_sb[:], in_=msk)
            bq_sb = constp.tile([128, 8], dt.float32, name="bq_sb")
            nc.sync.dma_start(out=bq_sb[:], in_=bqt)
            bk_sb = constp.tile([128, 8], dt.float32, name="bk_sb")
            nc.sync.dma_start(out=bk_sb[:], in_=bkt)
            bo_sb = constp.tile([128, 8], dt.float32, name="bo_sb")
            nc.sync.dma_start(out=bo_sb[:], in_=bot)
            ones1 = constp.tile([1, 64], dt.bfloat16, name="ones1")
            nc.vector.memset(ones1[:], 1.0)

            for _rep in range(repeat):
                import contextlib

                ctx_stack = contextlib.ExitStack()
                # ---- input DMAs, in phase-0 consumption order
                xts, wqs, wks, wvs, wos = [], [], [], [], []
                for k in range(8):
                    w = xwp.tile([128, D], dt.bfloat16, name=f"wq{k}", tag=f"wq{k}")
                    nc.sync.dma_start(out=w[:], in_=wq[k * 128 : (k + 1) * 128, :])
                    wqs.append(w)
                    t = xwp.tile([128, SEQ], dt.bfloat16, name=f"xt{k}", tag=f"xt{k}")
                    nc.sync.dma_start(out=t[:], in_=xT[k * 128 : (k + 1) * 128, :])
                    xts.append(t)
                for nm, wap, lst in (("wk", wk, wks), ("wv", wv, wvs), ("wo", wo, wos)):
                    for k in range(8):
                        w = xwp.tile(
                            [128, D], dt.bfloat16, name=f"{nm}{k}", tag=f"{nm}{k}"
                        )
                        nc.sync.dma_start(out=w[:], in_=wap[k * 128 : (k + 1) * 128, :])
                        lst.append(w)

                # per-pair SBUF tile allocators -------------------------------
                def alloc_qk(p):
                    qts = [
                        qkvp.tile([128, 512], dt.bfloat16, name=f"q{m}", tag=f"q{m}")
                        for m in range(8)
                    ]
                    kts = [
                        qkvp.tile([128, 512], dt.bfloat16, name=f"k{m}", tag=f"k{m}")
                        for m in range(8)
                    ]
                    return qts, kts

                def alloc_v(p):
                    vts = []
                    for st in range(4):
                        vt = qkvp.tile(
                            [128, NH, 65], dt.bfloat16, name=f"v{st}", tag=f"v{st}"
                        )
                        nc.vector.memset(vt[:, :, 64:65], 1.0)
                        vts.append(vt)
                    return vts

                def alloc_ats(p):
                    # 8 feature-chunk tiles per pair, cols = 512 pair positions
                    return [
                        atp.tile([128, 512], dt.bfloat16, name=f"at{k}", tag=f"at{k}")
                        for k in range(8)
                    ]

                # dense-work FIFOs: hard = QKV (schedule-critical),
                # soft = output projections (deferrable fillers)
                fifo_hard = []
                fifo_soft = []

                psum_pools = ctx_stack.enter_context(_psum_pools(tc))
                pdense, ps_s, ps_o, ps_r = psum_pools

                def push_qkv(p, qts, kts, vts):
                    pc0 = p * 512

                    def qk_thunk(wlist, b_sb, dst, m):
                        def run():
                            ps = pdense.tile(
                                [128, 512], dt.float32, name=f"pd_{m}", tag="dense"
                            )
                            for k in range(8):
                                nc.tensor.matmul(
                                    ps[:],
                                    wlist[k][:, m * 128 : (m + 1) * 128],
                                    xts[k][:, pc0 : pc0 + 512],
                                    start=(k == 0),
                                    stop=(k == 7),
                                )
                            nc.scalar.activation(
                                dst[m][:], ps[:], AF.Identity, bias=b_sb[:, m : m + 1]
                            )

                        return run

                    def v_thunk(st, ch):
                        def run():
                            ps = pdense.tile(
                                [128, 512], dt.float32, name=f"pv_{st}{ch}", tag="dense"
                            )
                            for k in range(8):
                                nc.tensor.matmul(
                                    ps[:],
                                    xts[k][:, pc0 + st * 128 : pc0 + (st + 1) * 128],
                                    wvs[k][:, ch * 512 : (ch + 1) * 512],
                                    start=(k == 0),
                                    stop=(k == 7),
                                )
                            nc.vector.tensor_copy(
                                vts[st][:, ch * 8 : (ch + 1) * 8, 0:64],
                                ps[:].rearrange("p (h c) -> p h c", h=8),
                            )

                        return run

                    for m in range(8):
                        fifo_hard.append(qk_thunk(wqs, bq_sb, qts, m))
                    for m in range(8):
                        fifo_hard.append(qk_thunk(wks, bk_sb, kts, m))
                    for st in range(4):
                        for ch in range(2):
                            fifo_hard.append(v_thunk(st, ch))

                def emit_op(p, ats, m):
                    pc0 = p * 512
                    ps = pdense.tile(
                        [128, 512], dt.float32, name=f"po_{m}", tag="dense"
                    )
                    for k in range(8):
                        nc.tensor.matmul(
                            ps[:],
                            wos[k][:, m * 128 : (m + 1) * 128],
                            ats[k][:],
                            start=(k == 0),
                            stop=(k == 7),
                        )
                    yt = yp.tile([128, 512], dt.bfloat16, name=f"yt{m}", tag="yt")
                    nc.vector.tensor_scalar_add(yt[:], ps[:], bo_sb[:, m : m + 1])
                    nc.sync.dma_start(
                        out=yT[m * 128 : (m + 1) * 128, pc0 : pc0 + 512],
                        in_=yt[:],
                    )

                def push_op(p, ats):
                    def op_thunk(m):
                        return lambda: emit_op(p, ats, m)

                    for m in range(8):
                        fifo_soft.append(op_thunk(m))

                def pop(reserve=0):
                    if fifo_hard:
                        fifo_hard.pop(0)()
                    elif len(fifo_soft) > reserve:
                        fifo_soft.pop(0)()

                # ---- phase 0: k-outer QKV for pair 0 (fills the DMA ramp).
                # PSUM slots are bank-granular, so phase-0 borrows one
                # generation of every main-pool tag (2+3+2+1 = 8 banks).
                qk0 = alloc_qk(0)
                vts0 = alloc_v(0)

                def alloc8(nm):
                    return [
                        pdense.tile([128, 512], dt.float32, name=f"{nm}d0", tag="dense"),
                        pdense.tile([128, 512], dt.float32, name=f"{nm}d1", tag="dense"),
                        ps_s.tile([128, 512], dt.float32, name=f"{nm}s0", tag="s"),
                        ps_s.tile([128, 512], dt.float32, name=f"{nm}s1", tag="s"),
                        ps_s.tile([128, 512], dt.float32, name=f"{nm}s2", tag="s"),
                        ps_o.tile([128, 512], dt.float32, name=f"{nm}o0", tag="o"),
                        ps_o.tile([128, 512], dt.float32, name=f"{nm}o1", tag="o"),
                        ps_r.tile([128, 512], dt.float32, name=f"{nm}r0", tag="r"),
                    ]

                if True:
                    for wlist, b_sb, dst in ((wqs, bq_sb, qk0[0]), (wks, bk_sb, qk0[1])):
                        pss = alloc8("q" if wlist is wqs else "k")
                        for k in range(8):
                            for m in range(8):
                                nc.tensor.matmul(
                                    pss[m][:],
                                    wlist[k][:, m * 128 : (m + 1) * 128],
                                    xts[k][:, 0:512],
                                    start=(k == 0),
                                    stop=(k == 7),
                                )
                        for m in range(8):
                            nc.scalar.activation(
                                dst[m][:],
                                pss[m][:],
                                AF.Identity,
                                bias=b_sb[:, m : m + 1],
                            )
                    pss = alloc8("v")
                    for k in range(8):
                        for st in range(4):
                            for ch in range(2):
                                nc.tensor.matmul(
                                    pss[st * 2 + ch][:],
                                    xts[k][:, st * 128 : (st + 1) * 128],
                                    wvs[k][:, ch * 512 : (ch + 1) * 512],
                                    start=(k == 0),
                                    stop=(k == 7),
                                )
                    for st in range(4):
                        for ch in range(2):
                            nc.vector.tensor_copy(
                                vts0[st][:, ch * 8 : (ch + 1) * 8, 0:64],
                                pss[st * 2 + ch][:].rearrange("p (h c) -> p h c", h=8),
                            )

                # ---- main loop
                if True:
                    cur_qk, cur_v = qk0, vts0
                    for p in range(N_PAIRS):
                        if p < N_PAIRS - 1:
                            nqk = alloc_qk(p + 1)
                            nv = alloc_v(p + 1)
                            push_qkv(p + 1, nqk[0], nqk[1], nv)
                        qts, kts = cur_qk
                        vts = cur_v
                        ats = alloc_ats(p)

                        for beta in range(2):
                            b = 2 * p + beta
                            bc0 = beta * 256
                            v0 = vts[2 * beta]
                            v1 = vts[2 * beta + 1]

                            # software-pipelined heads: S(t) ... PV(t-1)
                            es = {}

                            def s_phase(h):
                                ht, hp = h // 2, (h % 2) * 64
                                s_ = ps_s.tile(
                                    [128, 384], dt.float32, name=f"s{h}", tag="s"
                                )
                                nc.tensor.matmul(
                                    s_[:, 0:256],
                                    kts[ht][hp : hp + 64, bc0 : bc0 + 128],
                                    qts[ht][hp : hp + 64, bc0 : bc0 + 256],
                                    start=True,
                                    stop=True,
                                )
                                nc.tensor.matmul(
                                    s_[:, 256:384],
                                    kts[ht][hp : hp + 64, bc0 + 128 : bc0 + 256],
                                    qts[ht][hp : hp + 64, bc0 + 128 : bc0 + 256],
                                    start=True,
                                    stop=True,
                                )
                                e_ = attnp.tile(
                                    [128, 384], dt.bfloat16, name=f"e{h}", tag="e",
                                    bufs=4,
                                )
                                nc.scalar.activation(
                                    e_[:], s_[:], AF.Exp, scale=0.125
                                )
                                # zero the causally-invalid region (bf16 2x DVE)
                                nc.vector.tensor_tensor(
                                    e_[:], e_[:], mask_sb[:], mybir.AluOpType.mult
                                )
                                es[h] = e_

                            def pv_phase(t):
                                # both heads (2t, 2t+1) share one [65,512]
                                # PSUM bank; row 64 = softmax denominators
                                o2 = ps_o.tile(
                                    [65, 512], dt.float32, name=f"o{t}", tag="o"
                                )
                                for j in range(2):
                                    h = 2 * t + j
                                    e_ = es.pop(h)
                                    c0 = j * 256
                                    nc.tensor.matmul(
                                        o2[:, c0 : c0 + 128],
                                        v0[:, h, :],
                                        e_[:, 0:128],
                                        start=True,
                                        stop=True,
                                    )
                                    nc.tensor.matmul(
                                        o2[:, c0 + 128 : c0 + 256],
                                        v0[:, h, :],
                                        e_[:, 128:256],
                                        start=True,
                                        stop=False,
                                    )
                                    nc.tensor.matmul(
                                        o2[:, c0 + 128 : c0 + 256],
                                        v1[:, h, :],
                                        e_[:, 256:384],
                                        start=False,
                                        stop=True,
                                    )
                                # rr = 1/dsum as exp(-ln(dsum)) on ACT, both
                                # heads in one [1,512] op pair
                                lnv = attnp.tile(
                                    [1, 512], dt.float32, name="lnv", tag="lnv"
                                )
                                nc.scalar.activation(lnv[:], o2[64:65, :], AF.Ln)
                                rr = attnp.tile(
                                    [1, 512], dt.bfloat16, name="rr", tag="rr"
                                )
                                nc.scalar.activation(rr[:], lnv[:], AF.Exp, scale=-1.0)
                                r2 = ps_r.tile(
                                    [128, 256], dt.float32, name=f"r2_{t}", tag="r"
                                )
                                nc.tensor.matmul(
                                    r2[0:64, :], ones1[:], rr[:, 0:256],
                                    start=True, stop=True,
                                )
                                nc.tensor.matmul(
                                    r2[64:128, :], ones1[:], rr[:, 256:512],
                                    start=True, stop=True,
                                )
                                r2sb = attnp.tile(
                                    [128, 256], dt.float32, name=f"r2sb{t}", tag="r2sb"
                                )
                                nc.vector.tensor_copy(r2sb[:], r2[:])
                                for j in range(2):
                                    nc.vector.tensor_tensor(
                                        ats[t][j * 64 : (j + 1) * 64, bc0 : bc0 + 256],
                                        o2[0:64, j * 256 : (j + 1) * 256],
                                        r2sb[j * 64 : (j + 1) * 64, :],
                                        mybir.AluOpType.mult,
                                    )

                            rsv = 8 if b < 2 * N_PAIRS - 2 else 0
                            for t in range(9):
                                if t < 8:
                                    s_phase(2 * t)
                                    s_phase(2 * t + 1)
                                pop(rsv)
                                if t >= 1:
                                    pv_phase(t - 1)
                                pop(rsv)

                        push_op(p, ats)
                        cur_qk, cur_v = (nqk, nv) if p < N_PAIRS - 1 else (None, None)

                    while fifo_hard or fifo_soft:
                        pop()
                ctx_stack.close()
    if legalize:
        _legalize_waits(nc)
    return nc


def get_nc():
    if "nc" not in _cache:
        _cache["nc"] = _build_nc()
    return _cache["nc"]


def make_in_maps(x, Wq, bq, Wk, bk, Wv, bv, Wo, bo):
    """Host-side sharding/packing. Returns list of 8 per-core input dicts."""
    x = np.asarray(x, np.float32)
    Wq, Wk, Wv, Wo = (np.asarray(w, np.float32) for w in (Wq, Wk, Wv, Wo))
    bq, bk, bv, bo = (np.asarray(b, np.float32) for b in (bq, bk, bv, bo))

    # softmax rows sum to 1 -> attn @ (V + bv) = attn @ V + bv; fold into bo
    bo2 = bo + bv @ Wo

    wq_b, wk_b, wv_b, wo_b = (w.astype(BF16) for w in (Wq, Wk, Wv, Wo))
    bqt = np.ascontiguousarray(bq.reshape(8, 128).T)
    bkt = np.ascontiguousarray(bk.reshape(8, 128).T)
    bot = np.ascontiguousarray(bo2.reshape(8, 128).T)
    # S^T layout: rows = k, cols = q; invalid where k > q (strictly below
    # diag). 0/1 multiplicative mask applied to E after exp, bf16.
    tri01 = np.triu(np.ones((128, 128), np.float32))
    msk = np.concatenate(
        [tri01, np.ones((128, 128), np.float32), tri01], axis=1
    ).astype(BF16)

    xb = x.reshape(N_BLOCKS, BLK, D)
    in_maps = []
    for c in range(N_CORES):
        xc = xb[c * BLOCKS_PER_CORE : (c + 1) * BLOCKS_PER_CORE].reshape(SEQ, D)
        xTc = np.ascontiguousarray(xc.T.astype(BF16))
        in_maps.append(
            {
                "xT": xTc,
                "wq": wq_b,
                "wk": wk_b,
                "wv": wv_b,
                "wo": wo_b,
                "bqt": bqt,
                "bkt": bkt,
                "bot": bot,
                "msk": msk,
            }
        )
    return in_maps


def assemble_output(yT_list):
    """yT_list: per-core [1024, 2048] bf16 -> full [4, 4096, 1024] f32."""
    y = np.empty((N_BLOCKS, BLK, D), np.float32)
    for c in range(N_CORES):
        y[c * BLOCKS_PER_CORE : (c + 1) * BLOCKS_PER_CORE] = (
            np.asarray(yT_list[c], np.float32).T.reshape(BLOCKS_PER_CORE, BLK, D)
        )
    return np.ascontiguousarray(y.reshape(B, S, D))


def kernel(x, Wq, bq, Wk, bk, Wv, bv, Wo, bo):
    from concourse.bass_utils import run_bass_kernel_spmd

    in_maps = make_in_maps(x, Wq, bq, Wk, bk, Wv, bv, Wo, bo)
    nc = get_nc()
    res = run_bass_kernel_spmd(nc, in_maps, list(range(N_CORES)))
    return assemble_output([res.results[c]["yT"] for c in range(N_CORES)])
